# revision 6
# baseline (speedup 1.0000x reference)
"""CBANet on 8 Trainium2 NeuronCores (Bass/Tile).

Sharding: sample s = core//4 (B=2), each sample's N=12000 points split 4-way
(3000/core); the M=6000 selected-point stage reuses the same 4-core groups.

Math notes (vs reference.py):
  - 1x1 convs are matmuls over the point axis.
  - The broadcast global feature g only enters the three 1344-in decoders, so
    W @ [h1;h2;h3;g] = W[:, :320] @ [h1;h2;h3] + (W[:, 320:] @ g), where the
    second term is a per-sample bias vector -> contraction shrinks 1344->320.
  - EdgeConv max_k relu(W @ [xj-xi; xi] + b)
      = relu(max_k (Wa @ x)_j + ((Wb - Wa) @ x)_i + b),  W = [Wa | Wb],
    so it is a k-neighbor max-pool of A = Wa @ x plus a pointwise term.
  - fs (pointwise EdgeConv output) is only ever read at the Q=64 FPS points,
    so kNN/top-16/gather-max run for 64 rows only.
Host does only index/control glue: top-k ordering of dm, the sequential FPS
loop, gathers by index, and input sharding/weight transposes.
"""

import numpy as np
from functools import lru_cache

import concourse.bass as bass
import concourse.bacc as bacc
import concourse.mybir as mybir
from concourse import tile
from concourse import bass_utils
from concourse import library_config as libcfg

F32 = mybir.dt.float32
I16 = mybir.dt.int16
U32 = mybir.dt.uint32
AF = mybir.ActivationFunctionType
ALU = mybir.AluOpType
AX = mybir.AxisListType

B = 2
N = 12000
NC = N // 4          # 3000 points per core
M = N // 2           # 6000 selected points per sample
Q = 64
K = 16
C_OUT = 17
NCH = 500            # matmul column chunk (PSUM bank limit 512 fp32)
NEG_INF = -1.0e30


def _ch_tiles(C):
    out, c = [], 0
    while c < C:
        s = min(128, C - c)
        out.append((c, s))
        c += s
    return out


def _make_nc():
    return bacc.Bacc("TRN2", target_bir_lowering=False, debug=False,
                     num_devices=8)


class _L:
    """Common helpers for one launch build."""

    def __init__(self, nc, tc, ctx):
        self.nc, self.tc = nc, tc
        self.const = ctx.enter_context(tc.tile_pool(name="const", bufs=1))
        self.io = ctx.enter_context(tc.tile_pool(name="io", bufs=3))
        self.ps = ctx.enter_context(
            tc.tile_pool(name="ps", bufs=4, space=bass.MemorySpace.PSUM))
        self.ps2 = ctx.enter_context(
            tc.tile_pool(name="ps2", bufs=2, space=bass.MemorySpace.PSUM))
        self.ins = {}
        self.outs = {}

    def dram_in(self, name, shape):
        t = self.nc.dram_tensor(name, shape, F32, kind="ExternalInput")
        self.ins[name] = shape
        return t

    def dram_out(self, name, shape):
        t = self.nc.dram_tensor(name, shape, F32, kind="ExternalOutput")
        self.outs[name] = shape
        return t

    def load_const(self, dram, shape, tag):
        t = self.const.tile(list(shape), F32, tag=tag)
        self.nc.sync.dma_start(t[:], dram.ap())
        return t

    def load_w(self, name, cin, cout, tag=None):
        """Weight W^T [cin, cout] as list of (ktile, rows<=128)."""
        d = self.dram_in(name, [cin, cout])
        tiles = []
        for i, (co, cs) in enumerate(_ch_tiles(cin)):
            t = self.const.tile([cs, cout], F32, tag=f"{tag or name}.{i}")
            self.nc.sync.dma_start(t[:], d.ap()[co:co + cs, :])
            tiles.append(t)
        return tiles

    def load_b(self, name, cout, tag=None):
        """Bias [cout, 1] as list of per-outtile [s,1] bias tiles."""
        d = self.dram_in(name, [cout, 1])
        tiles = []
        for i, (co, cs) in enumerate(_ch_tiles(cout)):
            t = self.const.tile([cs, 1], F32, tag=f"{tag or name}.{i}")
            self.nc.sync.dma_start(t[:], d.ap()[co:co + cs, :])
            tiles.append(t)
        return tiles

    def linear(self, x_tiles, cin, cout, ncol, wk, bias, act, out_dtype=F32,
               out_tag=None):
        """y = act(W @ x + b).

        x_tiles: input ktiles [(tile, rows)] matching _ch_tiles(cin);
        wk: weight ktiles from load_w; bias: list per outtile (AP tiles) or
        None; act: 'relu' | 'sigmoid' | 'ident' | 'copy'.
        Returns list of output tiles [s, ncol] per _ch_tiles(cout).
        """
        nc = self.nc
        kt = _ch_tiles(cin)
        assert len(kt) == len(wk) == len(x_tiles)
        outs = []
        for oi, (oo, os_) in enumerate(_ch_tiles(cout)):
            ot = self.const.tile([os_, ncol], out_dtype,
                                 tag=out_tag and f"{out_tag}.{oi}")
            for nci in range(0, ncol, NCH):
                w = min(NCH, ncol - nci)
                pt = self.ps.tile([os_, NCH], F32, tag="mm")
                for ki, (ko, ks) in enumerate(kt):
                    nc.tensor.matmul(
                        pt[:, :w],
                        wk[ki][:, oo:oo + os_],
                        x_tiles[ki][:, nci:nci + w],
                        start=(ki == 0), stop=(ki == len(kt) - 1))
                dst = ot[:, nci:nci + w]
                if act == "relu":
                    nc.scalar.activation(dst, pt[:, :w], AF.Relu,
                                         bias=bias[oi][:, :])
                elif act == "sigmoid":
                    nc.scalar.activation(dst, pt[:, :w], AF.Sigmoid,
                                         bias=bias[oi][:, :])
                elif act == "ident":
                    nc.scalar.activation(dst, pt[:, :w], AF.Identity,
                                         bias=bias[oi][:, :])
                else:
                    nc.vector.tensor_copy(dst, pt[:, :w])
            outs.append(ot)
        return outs

    def store(self, dram, tiles, cout, ncol):
        for (oo, os_), t in zip(_ch_tiles(cout), tiles):
            self.nc.sync.dma_start(dram.ap()[oo:oo + os_, :], t[:])


def _build_l1():
    """Backbone convs + per-core max of the global-feature pre-activation."""
    nc = _make_nc()
    from contextlib import ExitStack
    with tile.TileContext(nc) as tc, ExitStack() as ctx:
        L = _L(nc, tc, ctx)
        x_d = L.dram_in("x", [15, NC])
        w1 = L.load_w("w1T", 15, 64)
        b1 = L.load_b("b1", 64)
        w2 = L.load_w("w2T", 64, 128)
        b2 = L.load_b("b2", 128)
        w3 = L.load_w("w3T", 128, 128)
        b3 = L.load_b("b3", 128)
        wg = L.load_w("wgT", 128, 1024)

        h1_d = L.dram_out("h1", [64, NC])
        h2_d = L.dram_out("h2", [128, NC])
        h3_d = L.dram_out("h3", [128, NC])
        gl_d = L.dram_out("gloc", [128, 8])

        x = L.load_const(x_d, [15, NC], "x")
        h1 = L.linear([x], 15, 64, NC, w1, b1, "relu", out_tag="h1")
        h2 = L.linear(h1, 64, 128, NC, w2, b2, "relu", out_tag="h2")
        h3 = L.linear(h2, 128, 128, NC, w3, b3, "relu", out_tag="h3")
        L.store(h1_d, h1, 64, NC)
        L.store(h2_d, h2, 128, NC)
        L.store(h3_d, h3, 128, NC)

        # gloc[:, j] = max_n (bbg_wT[:, 128j:...] ^T @ h3)[:, n]  (no relu/bias;
        # host applies relu(max + b) after the cross-core max).
        gl = L.const.tile([128, 8], F32, tag="gl")
        nchunks = NC // NCH
        for j in range(8):
            part = L.const.tile([128, nchunks], F32, tag="gpart")
            for ci in range(nchunks):
                pt = L.ps.tile([128, NCH], F32, tag="mm")
                nc.tensor.matmul(pt[:], wg[0][:, 128 * j:128 * (j + 1)],
                                 h3[0][:, ci * NCH:(ci + 1) * NCH],
                                 start=True, stop=True)
                nc.vector.tensor_reduce(part[:, ci:ci + 1], pt[:],
                                        axis=AX.X, op=ALU.max)
            nc.vector.tensor_reduce(gl[:, j:j + 1], part[:],
                                    axis=AX.X, op=ALU.max)
        nc.sync.dma_start(gl_d.ap(), gl[:])
    nc.compile()
    return nc, L.ins, L.outs


def _build_l2():
    """Decoders (320-chan contraction + g-matvec bias) + bmap/dmap heads +
    mask_feats."""
    nc = _make_nc()
    from contextlib import ExitStack
    with tile.TileContext(nc) as tc, ExitStack() as ctx:
        L = _L(nc, tc, ctx)
        h1_d = L.dram_in("h1", [64, NC])
        h2_d = L.dram_in("h2", [128, NC])
        h3_d = L.dram_in("h3", [128, NC])
        g_d = L.dram_in("g8", [128, 8])

        h1 = L.load_const(h1_d, [64, NC], "h1")
        h2 = L.load_const(h2_d, [128, NC], "h2")
        h3 = L.load_const(h3_d, [128, NC], "h3")
        g8 = L.load_const(g_d, [128, 8], "g8")
        feats = [h1, h2, h3]

        bmap_d = L.dram_out("bmap", [3, NC])
        dmap_d = L.dram_out("dmap", [1, NC])
        fd_d = L.dram_out("fd", [256, NC])
        mkf_d = L.dram_out("mkf", [256, NC])

        def decoder(pre):
            # weight tags shared across the three decoders (sequential use)
            wk = (L.load_w(f"{pre}_w1T", 64, 256, tag="dw1") +
                  L.load_w(f"{pre}_w2T", 128, 256, tag="dw2") +
                  L.load_w(f"{pre}_w3T", 128, 256, tag="dw3"))
            bb = L.load_b(f"{pre}_b", 256, tag="db")
            wg_d = L.dram_in(f"{pre}_wgT", [1024, 256])
            wg = L.io.tile([128, 8, 256], F32, tag="dwg")
            nc.sync.dma_start(
                wg[:], wg_d.ap().rearrange("(k p) o -> p k o", p=128))
            # bias vec = b + W[:,320:] @ g  per outtile
            bias = []
            for oi, (oo, os_) in enumerate(_ch_tiles(256)):
                pv = L.ps2.tile([os_, 1], F32, tag="vec")
                for k in range(8):
                    nc.tensor.matmul(pv[:], wg[:, k, oo:oo + os_],
                                     g8[:, k:k + 1],
                                     start=(k == 0), stop=(k == 7))
                bt = L.io.tile([os_, 1], F32, tag=f"dbias.{oi}")
                nc.vector.tensor_add(bt[:], pv[:], bb[oi][:])
                bias.append(bt)
            # all three decoder outputs share slots (used one at a time)
            return L.linear(feats, 320, 256, NC, wk, bias, "relu",
                            out_tag="dec")

        # dec -> fd -> mask_feats
        fd = decoder("dc")
        L.store(fd_d, fd, 256, NC)
        mk_w = L.load_w("mk_wT", 256, 256)
        mk_b = L.load_b("mk_b", 256)
        mkf = L.linear(fd, 256, 256, NC, mk_w, mk_b, "relu", out_tag="tA")
        L.store(mkf_d, mkf, 256, NC)

        # bmap head
        hb = decoder("bm")
        t1 = L.linear(hb, 256, 256, NC, L.load_w("bmh1_wT", 256, 256, tag="hw1"),
                      L.load_b("bmh1_b", 256, tag="hb1"), "relu", out_tag="tA")
        t2 = L.linear(t1, 256, 128, NC, L.load_w("bmh2_wT", 256, 128, tag="hw2"),
                      L.load_b("bmh2_b", 128, tag="hb2"), "relu", out_tag="tB")
        bm = L.linear(t2, 128, 3, NC, L.load_w("bmh3_wT", 128, 3, tag="hw3"),
                      L.load_b("bmh3_b", 3, tag="hb3"), "ident", out_tag="tC")
        L.store(bmap_d, bm, 3, NC)

        # dmap head
        hd = decoder("dm")
        u1 = L.linear(hd, 256, 256, NC, L.load_w("dmh1_wT", 256, 256, tag="hw1"),
                      L.load_b("dmh1_b", 256, tag="hb1"), "relu", out_tag="tA")
        u2 = L.linear(u1, 256, 128, NC, L.load_w("dmh2_wT", 256, 128, tag="hw2"),
                      L.load_b("dmh2_b", 128, tag="hb2"), "relu", out_tag="tB")
        dm = L.linear(u2, 128, 1, NC, L.load_w("dmh3_wT", 128, 1, tag="hw3"),
                      L.load_b("dmh3_b", 1, tag="hb3"), "ident", out_tag="tC")
        L.store(dmap_d, dm, 1, NC)
    nc.compile()
    return nc, L.ins, L.outs


def _build_l3():
    """kNN top-16 (Q=64 rows) + both EdgeConvs + cls/sc heads + mask decode."""
    nc = _make_nc()
    from contextlib import ExitStack
    with tile.TileContext(nc) as tc, ExitStack() as ctx:
        L = _L(nc, tc, ctx)
        fs_d = L.dram_in("fs", [256, M])          # fd gathered at t_idx
        gfs_d = L.dram_in("gfs", [256, Q])        # fs at sel (pre-EdgeConv)
        rhs4_d = L.dram_in("rhs4", [4, M])        # [2*ps; -|ps|^2]
        lhsT4_d = L.dram_in("lhsT4", [4, Q])      # [g_ps; 1]
        rhs4c_d = L.dram_in("rhs4c", [4, Q])      # [2*g_ps; -|g_ps|^2]
        mkf_d = L.dram_in("mkf", [256, NC])
        eye_d = L.dram_in("eye64", [64, 64])

        spm_d = L.dram_out("spm", [64, NC])
        spp_d = L.dram_out("spp", [C_OUT, Q])
        sco_d = L.dram_out("sco", [1, Q])

        nc.gpsimd.load_library(libcfg.ap_gather)
        eye = L.load_const(eye_d, [64, 64], "eye")

        def edgeconv(tag, a_src_tiles, cin, nelems, waT, wdT, wb, x_at_i,
                     lhsT4, rhs4):
            """Shared EdgeConv block: A table + kNN(top16) + gather-max.

            a_src_tiles: ktiles of the table input (A = Wa @ src, [256, nelems])
            x_at_i: ktiles [256, Q] of features at the 64 output rows.
            Returns list of [128, Q] output tiles (the EdgeConv output).
            """
            # A = Wa @ src  -> SBUF [128, nelems] x2 (DVE copy evict)
            A = L.linear(a_src_tiles, cin, 256, nelems, waT, None, "copy",
                         out_tag=f"{tag}_A")
            # negS = lhsT4^T @ rhs4  [64, nelems]
            ns = L.const.tile([64, nelems], F32, tag=f"{tag}_ns")
            for nci in range(0, nelems, NCH):
                w = min(NCH, nelems - nci)
                pt = L.ps.tile([64, NCH], F32, tag="mm")
                nc.tensor.matmul(pt[:, :w], lhsT4[:], rhs4[:, nci:nci + w],
                                 start=True, stop=True)
                nc.vector.tensor_copy(ns[:, nci:nci + w], pt[:, :w])
            # top-16 per row -> idx [64, 16] (two rounds of top-8)
            m1 = L.const.tile([64, 8], F32, tag=f"{tag}_m1")
            m2 = L.const.tile([64, 8], F32, tag=f"{tag}_m2")
            idxf = L.const.tile([64, 16], F32, tag=f"{tag}_idxf")
            i1 = L.const.tile([64, 8], U32, tag=f"{tag}_i1")
            i2 = L.const.tile([64, 8], U32, tag=f"{tag}_i2")
            nc.vector.max(m1[:], ns[:])
            nc.vector.max_index(i1[:], m1[:], ns[:])
            nc.vector.match_replace(ns[:], m1[:], ns[:], NEG_INF)
            nc.vector.max(m2[:], ns[:])
            nc.vector.max_index(i2[:], m2[:], ns[:])
            nc.vector.tensor_copy(idxf[:, 0:8], i1[:])
            nc.vector.tensor_copy(idxf[:, 8:16], i2[:])
            # transpose [64,16] -> [16,64], cast int16, replicate to 128 parts
            pT = L.ps2.tile([16, 64], F32, tag="tr")
            nc.tensor.transpose(pT[:], idxf[:], eye[:])
            ixT = L.const.tile([16, 64], I16, tag=f"{tag}_ixT")
            nc.vector.tensor_copy(ixT[:], pT[:])
            ix128 = L.const.tile([128, 64], I16, tag=f"{tag}_ix128")
            for ggi in range(8):
                nc.sync.dma_start(ix128[16 * ggi:16 * (ggi + 1), :], ixT[:])
            # gather + 16-group max + pointwise term
            outs = []
            for oi, (oo, os_) in enumerate(_ch_tiles(256)):
                gat = L.const.tile([128, Q * K], F32, tag=f"gat.{oi}")
                nc.gpsimd.ap_gather(gat[:], A[oi][:], ix128[:],
                                    channels=128, num_elems=nelems, d=1,
                                    num_idxs=Q * K)
                mx = L.const.tile([128, Q], F32, tag=f"mx.{oi}")
                nc.vector.tensor_reduce(
                    mx[:], gat[:].rearrange("p (q k) -> p q k", k=K),
                    axis=AX.X, op=ALU.max)
                pv = L.ps2.tile([os_, Q], F32, tag="bv")
                for ki, (ko, ks) in enumerate(_ch_tiles(cin)):
                    nc.tensor.matmul(pv[:], wdT[ki][:, oo:oo + os_],
                                     x_at_i[ki][:],
                                     start=(ki == 0),
                                     stop=(ki == len(_ch_tiles(cin)) - 1))
                sm = L.const.tile([os_, Q], F32, tag=f"{tag}_sm.{oi}")
                nc.vector.tensor_add(sm[:], pv[:], mx[:os_, :])
                ot = L.const.tile([os_, Q], F32, tag=f"{tag}_o.{oi}")
                nc.scalar.activation(ot[:], sm[:], AF.Relu, bias=wb[oi][:, :])
                outs.append(ot)
            return outs

        fs = []
        for i, (co, cs) in enumerate(_ch_tiles(256)):
            t = L.const.tile([cs, M], F32, tag=f"fs.{i}")
            nc.sync.dma_start(t[:], fs_d.ap()[co:co + cs, :])
            fs.append(t)
        gfs = []
        for i, (co, cs) in enumerate(_ch_tiles(256)):
            t = L.const.tile([cs, Q], F32, tag=f"gfs.{i}")
            nc.sync.dma_start(t[:], gfs_d.ap()[co:co + cs, :])
            gfs.append(t)
        lhsT4 = L.load_const(lhsT4_d, [4, Q], "lhsT4")
        rhs4 = L.load_const(rhs4_d, [4, M], "rhs4")
        rhs4c = L.load_const(rhs4c_d, [4, Q], "rhs4c")

        pw_aT = L.load_w("pw_aT", 256, 256)
        pw_dT = L.load_w("pw_dT", 256, 256)
        pw_b = L.load_b("pw_b", 256)
        g_fs = edgeconv("pw", fs, 256, M, pw_aT, pw_dT, pw_b, gfs,
                        lhsT4, rhs4)

        cw_aT = L.load_w("cw_aT", 256, 256)
        cw_dT = L.load_w("cw_dT", 256, 256)
        cw_b = L.load_b("cw_b", 256)
        g_feats = edgeconv("cw", g_fs, 256, Q, cw_aT, cw_dT, cw_b, g_fs,
                           lhsT4, rhs4c)

        # cls / score heads on g_feats [256, 64]
        c1 = L.linear(g_feats, 256, 256, Q, L.load_w("cl1_wT", 256, 256),
                      L.load_b("cl1_b", 256), "relu", out_tag="c1")
        c2 = L.linear(c1, 256, 128, Q, L.load_w("cl2_wT", 256, 128),
                      L.load_b("cl2_b", 128), "relu", out_tag="c2")
        c3 = L.linear(c2, 128, C_OUT, Q, L.load_w("cl3_wT", 128, C_OUT),
                      L.load_b("cl3_b", C_OUT), "ident", out_tag="c3")
        L.store(spp_d, c3, C_OUT, Q)
        s1 = L.linear(g_feats, 256, 256, Q, L.load_w("sc1_wT", 256, 256),
                      L.load_b("sc1_b", 256), "relu", out_tag="s1")
        s2 = L.linear(s1, 256, 128, Q, L.load_w("sc2_wT", 256, 128),
                      L.load_b("sc2_b", 128), "relu", out_tag="s2")
        s3 = L.linear(s2, 128, 1, Q, L.load_w("sc3_wT", 128, 1),
                      L.load_b("sc3_b", 1), "ident", out_tag="s3")
        L.store(sco_d, s3, 1, Q)

        # mf = g_feats^T @ mask_feats  [64, NC]; sp_masks = sig(mh2@relu(mh1@mf))
        mkf = []
        for i, (co, cs) in enumerate(_ch_tiles(256)):
            # reuses the fs slots (fs is dead once A is built)
            t = L.const.tile([cs, NC], F32, tag=f"fs.{i}")
            nc.sync.dma_start(t[:], mkf_d.ap()[co:co + cs, :])
            mkf.append(t)
        mf = L.const.tile([64, NC], F32, tag="big1.0")
        for nci in range(0, NC, NCH):
            pt = L.ps.tile([64, NCH], F32, tag="mm")
            for ki in range(2):
                nc.tensor.matmul(pt[:], g_feats[ki][:], mkf[ki][:, nci:nci + NCH],
                                 start=(ki == 0), stop=(ki == 1))
            nc.vector.tensor_copy(mf[:, nci:nci + NCH], pt[:])
        v1 = L.linear([mf], 64, 64, NC, L.load_w("mh1_wT", 64, 64),
                      L.load_b("mh1_b", 64), "relu", out_tag="v1")
        spm = L.linear(v1, 64, 64, NC, L.load_w("mh2_wT", 64, 64),
                       L.load_b("mh2_b", 64), "sigmoid", out_tag="big1")
        L.store(spm_d, spm, 64, NC)
    nc.compile()
    return nc, L.ins, L.outs


@lru_cache(maxsize=None)
def _modules():
    return {"l1": _build_l1(), "l2": _build_l2(), "l3": _build_l3()}


def _run(mod, in_maps, **kw):
    nc, ins, outs = mod
    for m in in_maps:
        assert set(m) == set(ins), (sorted(m), sorted(ins))
        for k, v in m.items():
            assert list(v.shape) == list(ins[k]), (k, v.shape, ins[k])
            assert v.dtype == np.float32 or k.startswith("ix"), (k, v.dtype)
    res = bass_utils.run_bass_kernel_spmd(nc, in_maps, core_ids=list(range(8)),
                                          **kw)
    return res.results


def _wT(w):
    return np.ascontiguousarray(np.asarray(w, np.float32).T)


def _b1(b):
    return np.ascontiguousarray(np.asarray(b, np.float32).reshape(-1, 1))


def _fps_host(pts, q):
    """pts [M,3] f32; seed = argmax(dm_sel) = 0 (dm_sel sorted desc)."""
    sel = np.empty(q, np.int32)
    sel[0] = 0
    d = ((pts - pts[0]) ** 2).sum(-1, dtype=np.float32)
    for t in range(1, q):
        i = int(np.argmax(d))
        sel[t] = i
        d = np.minimum(d, ((pts - pts[i]) ** 2).sum(-1, dtype=np.float32))
    return sel


def kernel(x, params, eid):
    x = np.asarray(x, np.float32)
    eid = int(np.asarray(eid))
    P = {k: {"W": np.asarray(v["W"], np.float32),
             "b": np.asarray(v["b"], np.float32)} for k, v in params.items()}
    mods = _modules()

    def chunk(arr2d, c):
        # core c -> sample c//4, columns [(c%4)*NC, ...)
        s, qd = divmod(c, 4)
        return np.ascontiguousarray(arr2d[s][:, qd * NC:(qd + 1) * NC])

    # ---- launch 1
    in1 = []
    w_shared1 = {
        "w1T": _wT(P["bb1"]["W"]), "b1": _b1(P["bb1"]["b"]),
        "w2T": _wT(P["bb2"]["W"]), "b2": _b1(P["bb2"]["b"]),
        "w3T": _wT(P["bb3"]["W"]), "b3": _b1(P["bb3"]["b"]),
        "wgT": _wT(P["bbg"]["W"]),
    }
    for c in range(8):
        in1.append({"x": chunk(x, c), **w_shared1})
    r1 = _run(mods["l1"], in1)

    # host: g = relu(max over cores + bbg_b) per sample  [1024]
    g = np.empty((B, 1024), np.float32)
    for s in range(B):
        gl = np.max([r1[4 * s + q]["gloc"] for q in range(4)], axis=0)
        g[s] = np.maximum(gl.T.reshape(-1) + P["bbg"]["b"], 0.0)

    # ---- launch 2
    w_shared2 = {}
    for pre, nm in (("bmap_dec", "bm"), ("dmap_dec", "dm"), ("dec", "dc")):
        W = P[pre]["W"]
        w_shared2[f"{nm}_w1T"] = _wT(W[:, :64])
        w_shared2[f"{nm}_w2T"] = _wT(W[:, 64:192])
        w_shared2[f"{nm}_w3T"] = _wT(W[:, 192:320])
        w_shared2[f"{nm}_wgT"] = _wT(W[:, 320:])
        w_shared2[f"{nm}_b"] = _b1(P[pre]["b"])
    for pre, nm in (("bmap_h", "bmh"), ("dmap_h", "dmh")):
        for i in (1, 2, 3):
            w_shared2[f"{nm}{i}_wT"] = _wT(P[f"{pre}{i}"]["W"])
            w_shared2[f"{nm}{i}_b"] = _b1(P[f"{pre}{i}"]["b"])
    w_shared2["mk_wT"] = _wT(P["mask_dec"]["W"])
    w_shared2["mk_b"] = _b1(P["mask_dec"]["b"])
    in2 = []
    for c in range(8):
        s = c // 4
        g8 = np.ascontiguousarray(g[s].reshape(8, 128).T)
        in2.append({"h1": r1[c]["h1"], "h2": r1[c]["h2"], "h3": r1[c]["h3"],
                    "g8": g8, **w_shared2})
    r2 = _run(mods["l2"], in2)

    bmap_out = np.stack(
        [np.concatenate([r2[4 * s + q]["bmap"] for q in range(4)], axis=1)
         for s in range(B)])
    dmap_out = np.stack(
        [np.concatenate([r2[4 * s + q]["dmap"] for q in range(4)], axis=1)
         for s in range(B)])

    if eid <= 19:
        return (bmap_out, dmap_out,
                np.zeros((B, N, Q), np.float32),
                np.zeros((B, C_OUT, Q), np.float32),
                np.zeros((B, 1, Q), np.float32),
                np.zeros((B, Q), np.int32))

    fd = np.stack(
        [np.concatenate([r2[4 * s + q]["fd"] for q in range(4)], axis=1)
         for s in range(B)])
    mkf = [np.ascontiguousarray(r2[c]["mkf"]) for c in range(8)]

    # ---- host: top-M ordering of dm, gathers, FPS
    dm = dmap_out[:, 0, :]                      # [B, N]
    t_idx = np.argsort(-dm, axis=-1, kind="stable")[:, :M].astype(np.int32)
    dm_sel = np.take_along_axis(dm, t_idx, axis=1)
    p = x[:, :3, :]
    ps = np.take_along_axis(p, t_idx[:, None, :].repeat(3, 1), axis=2)
    fs_pre = np.stack([fd[s][:, t_idx[s]] for s in range(B)])
    sel = np.stack([_fps_host(np.ascontiguousarray(ps[s].T), Q)
                    for s in range(B)])         # [B, Q]
    all_idx = np.take_along_axis(t_idx, sel, axis=1)
    g_ps = np.stack([ps[s][:, sel[s]] for s in range(B)])
    gfs_pre = np.stack([fs_pre[s][:, sel[s]] for s in range(B)])

    def dist4(pts):  # [3, n] -> lhs/rhs tiles for negS matmuls
        sq = (pts * pts).sum(0, dtype=np.float32)
        return np.concatenate([2.0 * pts, -sq[None, :]], 0).astype(np.float32)

    # ---- launch 3
    w_shared3 = {"eye64": np.eye(64, dtype=np.float32)}
    for pre, nm in (("pw", "pw"), ("cw", "cw")):
        W = P[pre]["W"]
        w_shared3[f"{nm}_aT"] = _wT(W[:, :256])
        w_shared3[f"{nm}_dT"] = _wT(W[:, 256:] - W[:, :256])
        w_shared3[f"{nm}_b"] = _b1(P[pre]["b"])
    for pre, nm in (("cls_h", "cl"), ("sc_h", "sc")):
        for i in (1, 2, 3):
            w_shared3[f"{nm}{i}_wT"] = _wT(P[f"{pre}{i}"]["W"])
            w_shared3[f"{nm}{i}_b"] = _b1(P[f"{pre}{i}"]["b"])
    for nm in ("mh1", "mh2"):
        w_shared3[f"{nm}_wT"] = _wT(P[nm]["W"])
        w_shared3[f"{nm}_b"] = _b1(P[nm]["b"])
    in3 = []
    for c in range(8):
        s = c // 4
        lt4 = np.concatenate([g_ps[s], np.ones((1, Q), np.float32)],
                             0).astype(np.float32)
        in3.append({
            "fs": fs_pre[s], "gfs": gfs_pre[s],
            "rhs4": dist4(ps[s]), "lhsT4": lt4, "rhs4c": dist4(g_ps[s]),
            "mkf": mkf[c], **w_shared3})
    r3 = _run(mods["l3"], in3)

    sp_masks = np.stack(
        [np.concatenate([r3[4 * s + q]["spm"] for q in range(4)], axis=1)
         for s in range(B)])
    sp_probs = np.stack([r3[4 * s]["spp"] for s in range(B)])
    score = np.stack([r3[4 * s]["sco"] for s in range(B)])
    return (bmap_out, dmap_out, sp_masks, sp_probs, score, all_idx)
